# revision 12
# baseline (speedup 1.0000x reference)
"""Causal self-attention (B=4, S=2048, D=1024, single head) on 8 TRN2 cores.

Sharding: core c handles batch b = c//2 with query-tile parity p = c%2 —
its 8 query tiles of 128 rows are the absolute 128-row tiles {2j+p}.
Interleaving parities balances causal work exactly: both cores of a pair
process the same kv extent per local tile j, so the single SPMD program
is uniform; per-core variation is data-only (query rows and the additive
causal mask for the last kv group).

All matmuls run in float32r (full-rate fp32 with reduced mantissa):
  qT[o,s] = WqT.T @ xT        kT likewise       v[s,o] = xT.T @ WvT
  scores[sq,kv] = qT.T @ kT (+ identity.T @ mask on the last group)
  P = exp(scale*scores) with fused row-sum (ScalarE accum_out)
  PT = PE-transpose(P);  out[sq,o] = PT.T @ v;  out *= 1/rowsum
"""

import numpy as np

B, S, D = 4, 2048, 1024
DC = D // 128          # contraction chunks
NB = S // 128          # kv blocks per batch
NT = 8                 # q tiles per core
SCALE = 1.0 / np.sqrt(np.float32(D))
NEG = np.float32(-1e30)

_cache = {}


def _ext(j):
    # kv extent for local tile j in 128-blocks (uniform across cores);
    # rounded up to a multiple of 2 so the tail group is 256-wide
    return 2 * j + 2


def _build():
    if "nc" in _cache:
        return _cache["nc"]

    import concourse.bacc as bacc
    import concourse.mybir as mybir
    import concourse.tile as tile

    f32 = mybir.dt.float32
    f32r = mybir.dt.float32r
    AF = mybir.ActivationFunctionType

    nc = bacc.Bacc("TRN2", target_bir_lowering=False, debug=False,
                   num_devices=8)
    xq_d = nc.dram_tensor("xq", [D, NT * 128], f32r, kind="ExternalInput").ap()
    xkv_d = nc.dram_tensor("xkv", [D, S], f32r, kind="ExternalInput").ap()
    w_d = {n: nc.dram_tensor(n, [D, D], f32r, kind="ExternalInput").ap()
           for n in ("wq", "wk", "wv")}
    masks_d = nc.dram_tensor("masks", [NT * 128, 512], f32r,
                             kind="ExternalInput").ap()  # col 0..gw(j) used
    ident_d = nc.dram_tensor("ident", [128, 128], f32r,
                             kind="ExternalInput").ap()
    out_d = nc.dram_tensor("out", [NT * 128, D], f32,
                           kind="ExternalOutput").ap()

    with tile.TileContext(nc) as tc:
        with (
            tc.tile_pool(name="persist", bufs=1) as persist,
            tc.tile_pool(name="dram", bufs=1, space="DRAM") as dramp,
        ):
            kT = persist.tile([128, DC * S], f32r)          # [o%128, oc*S+kv]
            qT = persist.tile([128, DC * NT * 128], f32r)   # [o%128, oc*1024+sq]
            vtmp = [dramp.tile([512, D], f32r, name=f"vtmp{i}",
                               tag=f"vtmp{i}") for i in range(4)]

            vsb0a = persist.tile([128, 2 * D], f32r)  # kv blocks 0..1
            with (
                tc.tile_pool(name="wpool", bufs=2) as wp,
                tc.tile_pool(name="xpool", bufs=2) as xs,
                tc.tile_pool(name="evpool", bufs=3) as ev,
                tc.tile_pool(name="pspj", bufs=4, space="PSUM") as psp,
            ):
                def load_w(name, cuts=(0, DC // 2, DC)):
                    w = wp.tile([128, DC * D], f32r, name=f"w_{name}",
                                tag="w")  # [d%128, dc*D + o]
                    src = w_d[name].rearrange("(c p) o -> p c o", p=128)
                    wv3 = w[:].rearrange("p (c o) -> p c o", c=DC)
                    for a, b in zip(cuts[:-1], cuts[1:]):
                        nc.sync.dma_start(wv3[:, a:b], src[:, a:b])
                    return w

                def load_x(src_ap, c0, cuts=(0, DC)):
                    xt = xs.tile([128, DC * 512], f32r, tag="x")
                    dst = xt[:].rearrange("p (c s) -> p c s", c=DC)
                    src = src_ap[:, c0 * 512:(c0 + 1) * 512] \
                        .rearrange("(c p) s -> p c s", p=128)
                    for a, b in zip(cuts[:-1], cuts[1:]):
                        nc.sync.dma_start(dst[:, a:b], src[:, a:b])
                    return xt

                # first-needed data first, in small pieces
                wq = load_w("wq", cuts=(0, 2, 4, DC))
                xt0 = load_x(xq_d, 0, cuts=(0, 2, 4, DC))
                xt1 = load_x(xq_d, 1)
                wk = load_w("wk")

                # ---- Phase A: q projection (into resident qT) ----
                for sg in range(2):
                    xt = xt0 if sg == 0 else xt1
                    for ot in range(8):
                        ps = psp.tile([128, 512], f32, tag="pj")
                        for dc in range(DC):
                            nc.tensor.matmul(
                                ps[:],
                                wq[:, dc * D + ot * 128:dc * D + ot * 128 + 128],
                                xt[:, dc * 512:(dc + 1) * 512],
                                start=(dc == 0), stop=(dc == DC - 1))
                        nc.vector.tensor_copy(
                            qT[:, ot * 1024 + sg * 512:ot * 1024 + sg * 512 + 512],
                            ps[:])

                wv = load_w("wv")  # takes wq's slot; prefetches during BC

                # ---- Phase BC: k and v projections from shared x chunks ----
                for sg in range(4):
                    xt = load_x(xkv_d, sg)
                    for ot in range(8):
                        ps = psp.tile([128, 512], f32, tag="pj")
                        for dc in range(DC):
                            nc.tensor.matmul(
                                ps[:],
                                wk[:, dc * D + ot * 128:dc * D + ot * 128 + 128],
                                xt[:, dc * 512:(dc + 1) * 512],
                                start=(dc == 0), stop=(dc == DC - 1))
                        nc.vector.tensor_copy(
                            kT[:, ot * S + sg * 512:ot * S + sg * 512 + 512],
                            ps[:])
                    for st in range(4):
                        for og in range(2):
                            ps = psp.tile([128, 512], f32, tag="pj")
                            for dc in range(DC):
                                nc.tensor.matmul(
                                    ps[:],
                                    xt[:, dc * 512 + st * 128:dc * 512 + st * 128 + 128],
                                    wv[:, dc * D + og * 512:dc * D + og * 512 + 512],
                                    start=(dc == 0), stop=(dc == DC - 1))
                            vtb = ev.tile([128, 512], f32r, tag="ev")
                            nc.vector.tensor_copy(vtb[:], ps[:])
                            nc.gpsimd.dma_start(
                                vtmp[sg][st * 128:st * 128 + 128,
                                         og * 512:og * 512 + 512],
                                vtb[:])
                    if sg == 0:
                        nc.sync.dma_start(
                            vsb0a[:].rearrange("p (c o) -> p c o", c=2),
                            vtmp[0][0:256, :]
                            .rearrange("(c p) o -> p c o", p=128))

            # ---- Phase D: attention ----
            with (
                tc.tile_pool(name="vD", bufs=1) as vp,
                tc.tile_pool(name="cD", bufs=1) as cp,
                tc.tile_pool(name="pD", bufs=2) as pp,
                tc.tile_pool(name="ptD", bufs=1) as ptp,
                tc.tile_pool(name="oD", bufs=1) as op,
                tc.tile_pool(name="smD", bufs=2) as smp,
                tc.tile_pool(name="psS", bufs=3, space="PSUM") as ps_s,
                tc.tile_pool(name="psT", bufs=2, space="PSUM") as ps_t,
                tc.tile_pool(name="psO", bufs=1, space="PSUM") as ps_o,
            ):
                ident = cp.tile([128, 128], f32r)
                nc.sync.dma_start(ident[:], ident_d)
                # vsb[0] holds only blocks 2..3 (0..1 live in vsb0a)
                vsb = [vp.tile([128, (2 if q4 == 0 else 4) * D], f32r,
                               name=f"vsb{q4}", tag=f"v{q4}")
                       for q4 in range(4)]

                def load_v(q4):
                    nb = 2 if q4 == 0 else 4
                    src_rows = vtmp[q4][512 - nb * 128:512, :] \
                        .rearrange("(c p) o -> p c o", p=128)
                    dst = vsb[q4][:].rearrange("p (c o) -> p c o", c=nb)
                    if q4 == 0:
                        for c in range(nb):
                            nc.sync.dma_start(dst[:, c], src_rows[:, c])
                    else:
                        nc.sync.dma_start(dst, src_rows)

                load_v(0)
                masks = cp.tile([128, NT * 512], f32r)  # [p, j*512+kv]
                nc.sync.dma_start(
                    masks[:].rearrange("p (j k) -> p j k", j=NT),
                    masks_d.rearrange("(j p) k -> p j k", p=128))
                for q4 in range(1, 4):
                    load_v(q4)

                def vblk(kb):
                    if kb < 2:
                        return vsb0a[:, kb * D:(kb + 1) * D]
                    if kb < 4:
                        return vsb[0][:, (kb - 2) * D:(kb - 1) * D]
                    return vsb[kb // 4][:, (kb % 4) * D:(kb % 4 + 1) * D]

                Ph, dsh, rch = {}, {}, {}

                def scores_exp(j):
                    ext = _ext(j)
                    ng = (ext + 3) // 4
                    P = pp.tile([128, NB * 128], f32r, tag="P",
                                name=f"P{j}")
                    dslots = smp.tile([128, 4], f32, tag="ds",
                                      name=f"ds{j}")
                    for g in range(ng):
                        gw = min(512, ext * 128 - g * 512)
                        last = (g == ng - 1)
                        sps = ps_s.tile([128, 512], f32, tag="sc",
                                        name=f"sps{j}_{g}")
                        for oc in range(DC):
                            nc.tensor.matmul(
                                sps[:, 0:gw],
                                qT[:, oc * 1024 + j * 128:oc * 1024 + j * 128 + 128],
                                kT[:, oc * S + g * 512:oc * S + g * 512 + gw],
                                start=(oc == 0),
                                stop=(oc == DC - 1 and not last))
                        if last:
                            nc.tensor.matmul(
                                sps[:, 0:gw], ident[:],
                                masks[:, j * 512:j * 512 + gw],
                                start=False, stop=True)
                        nc.scalar.activation(
                            P[:, g * 512:g * 512 + gw], sps[:, 0:gw], AF.Exp,
                            scale=float(SCALE),
                            accum_out=dslots[:, g:g + 1])
                    rcp = smp.tile([128, 1], f32, tag="rcp", name=f"rcp{j}")
                    den = smp.tile([128, 1], f32, tag="den", name=f"den{j}")
                    nc.vector.reduce_sum(den[:], dslots[:, 0:ng],
                                         axis=mybir.AxisListType.X)
                    nc.vector.reciprocal(rcp[:], den[:])
                    Ph[j], dsh[j], rch[j] = P, dslots, rcp

                def transp_av(j):
                    ext = _ext(j)
                    ng = (ext + 3) // 4
                    P, rcp = Ph[j], rch[j]
                    PT = ptp.tile([128, NB * 128], f32r, tag="PT",
                                  name=f"PT{j}")
                    for g in range(ng):
                        nb = min(4, ext - g * 4)
                        tps = ps_t.tile([128, 512], f32r, tag="tp",
                                        name=f"tps{j}_{g}")
                        for bb in range(nb):
                            nc.tensor.transpose(
                                tps[:, bb * 128:(bb + 1) * 128],
                                P[:, g * 512 + bb * 128:g * 512 + bb * 128 + 128],
                                ident[:])
                        nc.vector.tensor_copy(
                            PT[:, g * 512:g * 512 + nb * 128],
                            tps[:, 0:nb * 128])

                    ops = ps_o.tile([128, D], f32, tag="av", name=f"ops{j}")
                    for og in range(2):
                        for kb in range(ext):
                            nc.tensor.matmul(
                                ops[:, og * 512:(og + 1) * 512],
                                PT[:, kb * 128:(kb + 1) * 128],
                                vblk(kb)[:, og * 512:(og + 1) * 512],
                                start=(kb == 0), stop=(kb == ext - 1))
                    osb = op.tile([128, D], f32, tag="o", name=f"o{j}")
                    nc.vector.tensor_scalar_mul(osb[:], ops[:], rcp[:])
                    nc.sync.dma_start(out_d[j * 128:(j + 1) * 128, :], osb[:])

                scores_exp(0)
                for j in range(NT):
                    if j + 1 < NT:
                        scores_exp(j + 1)
                    transp_av(j)

    nc.compile()
    _cache["nc"] = nc
    return nc


def _shard(x, Wq, Wk, Wv):
    """Build the 8 per-core input maps from full inputs."""
    ident = np.eye(128, dtype=np.float32)
    wqt = np.ascontiguousarray(Wq.T)
    wkt = np.ascontiguousarray(Wk.T)
    wvt = np.ascontiguousarray(Wv.T)
    in_maps = []
    for c in range(8):
        b, p = c // 2, c % 2
        xb = x[b]
        xkv = np.ascontiguousarray(xb.T)
        rows = np.concatenate(
            [xb[(2 * j + p) * 128:(2 * j + p + 1) * 128, :] for j in range(8)],
            axis=0)
        xq = np.ascontiguousarray(rows.T)
        masks = np.full((NT * 128, 512), NEG, np.float32)
        for j in range(NT):
            ext = _ext(j)
            ng = (ext + 3) // 4
            gw = min(512, ext * 128 - (ng - 1) * 512)
            q_abs = (2 * j + p) * 128 + np.arange(128)[:, None]
            kv_abs = (ng - 1) * 512 + np.arange(gw)[None, :]
            masks[j * 128:(j + 1) * 128, 0:gw] = np.where(
                kv_abs <= q_abs, np.float32(0), NEG)
        in_maps.append({
            "xq": xq, "xkv": xkv, "wq": wqt, "wk": wkt, "wv": wvt,
            "masks": masks, "ident": ident,
        })
    return in_maps


def _unshard(results, dtype):
    out = np.empty((B, S, D), dtype)
    for c in range(8):
        b, p = c // 2, c % 2
        o = results[c]["out"]
        for j in range(NT):
            out[b, (2 * j + p) * 128:(2 * j + p + 1) * 128, :] = \
                o[j * 128:(j + 1) * 128, :]
    return out


def run(x, Wq, Wk, Wv, trace=False):
    from concourse.bass_utils import run_bass_kernel_spmd
    nc = _build()
    in_maps = _shard(np.asarray(x), np.asarray(Wq), np.asarray(Wk),
                     np.asarray(Wv))
    res = run_bass_kernel_spmd(nc, in_maps, core_ids=list(range(8)),
                               trace=trace)
    return _unshard(res.results, np.float32), res


def kernel(x, Wq, Wk, Wv):
    out, _ = run(x, Wq, Wk, Wv, trace=False)
    return out


# revision 13
# speedup vs baseline: 1.0082x; 1.0082x over previous
"""Causal self-attention (B=4, S=2048, D=1024, single head) on 8 TRN2 cores.

Sharding: core c handles batch b = c//2 with query-tile parity p = c%2 —
its 8 query tiles of 128 rows are the absolute 128-row tiles {2j+p}.
Interleaving parities balances causal work exactly: both cores of a pair
process the same kv extent per local tile j, so the single SPMD program
is uniform; per-core variation is data-only (query rows and the additive
causal mask for the last kv group).

All matmuls run in float32r (full-rate fp32 with reduced mantissa):
  qT[o,s] = WqT.T @ xT        kT likewise       v[s,o] = xT.T @ WvT
  scores[sq,kv] = qT.T @ kT (+ identity.T @ mask on the last group)
  P = exp(scale*scores) with fused row-sum (ScalarE accum_out)
  PT = PE-transpose(P);  out[sq,o] = PT.T @ v;  out *= 1/rowsum
"""

import numpy as np

B, S, D = 4, 2048, 1024
DC = D // 128          # contraction chunks
NB = S // 128          # kv blocks per batch
NT = 8                 # q tiles per core
SCALE = 1.0 / np.sqrt(np.float32(D))
NEG = np.float32(-1e30)

_cache = {}


def _ext(j):
    # kv extent for local tile j in 128-blocks (uniform across cores);
    # rounded up to a multiple of 2 so the tail group is 256-wide
    return 2 * j + 2


def _build():
    if "nc" in _cache:
        return _cache["nc"]

    import concourse.bacc as bacc
    import concourse.mybir as mybir
    import concourse.tile as tile

    f32 = mybir.dt.float32
    f32r = mybir.dt.float32r
    AF = mybir.ActivationFunctionType

    nc = bacc.Bacc("TRN2", target_bir_lowering=False, debug=False,
                   num_devices=8)
    xq_d = nc.dram_tensor("xq", [D, NT * 128], f32r, kind="ExternalInput").ap()
    xkv_d = nc.dram_tensor("xkv", [D, S], f32r, kind="ExternalInput").ap()
    w_d = {n: nc.dram_tensor(n, [D, D], f32r, kind="ExternalInput").ap()
           for n in ("wq", "wk", "wv")}
    masks_d = nc.dram_tensor("masks", [NT * 128, 512], f32r,
                             kind="ExternalInput").ap()  # col 0..gw(j) used
    ident_d = nc.dram_tensor("ident", [128, 128], f32r,
                             kind="ExternalInput").ap()
    out_d = nc.dram_tensor("out", [NT * 128, D], f32,
                           kind="ExternalOutput").ap()

    with tile.TileContext(nc) as tc:
        with (
            tc.tile_pool(name="persist", bufs=1) as persist,
            tc.tile_pool(name="dram", bufs=1, space="DRAM") as dramp,
        ):
            kT = persist.tile([128, DC * S], f32r)          # [o%128, oc*S+kv]
            qT = persist.tile([128, DC * NT * 128], f32r)   # [o%128, oc*1024+sq]
            vtmp = [dramp.tile([512, D], f32r, name=f"vtmp{i}",
                               tag=f"vtmp{i}") for i in range(4)]

            vsb0a = persist.tile([128, 2 * D], f32r)  # kv blocks 0..1
            with (
                tc.tile_pool(name="wpool", bufs=2) as wp,
                tc.tile_pool(name="xpool", bufs=2) as xs,
                tc.tile_pool(name="evpool", bufs=3) as ev,
                tc.tile_pool(name="pspj", bufs=4, space="PSUM") as psp,
            ):
                def load_w(name, cuts=(0, DC // 2, DC)):
                    w = wp.tile([128, DC * D], f32r, name=f"w_{name}",
                                tag="w")  # [d%128, dc*D + o]
                    src = w_d[name].rearrange("(c p) o -> p c o", p=128)
                    wv3 = w[:].rearrange("p (c o) -> p c o", c=DC)
                    for a, b in zip(cuts[:-1], cuts[1:]):
                        nc.sync.dma_start(wv3[:, a:b], src[:, a:b])
                    return w

                def load_x(src_ap, c0, cuts=(0, DC)):
                    xt = xs.tile([128, DC * 512], f32r, tag="x")
                    dst = xt[:].rearrange("p (c s) -> p c s", c=DC)
                    src = src_ap[:, c0 * 512:(c0 + 1) * 512] \
                        .rearrange("(c p) s -> p c s", p=128)
                    for a, b in zip(cuts[:-1], cuts[1:]):
                        nc.sync.dma_start(dst[:, a:b], src[:, a:b])
                    return xt

                warm = ev.tile([128, 1], f32, tag="warm")
                nc.gpsimd.memset(warm[:], 0.0)
                nc.scalar.activation(warm[:], warm[:], AF.Exp)

                # first-needed data first, in small pieces
                wq = load_w("wq", cuts=(0, 2, 4, DC))
                xt0 = load_x(xq_d, 0, cuts=(0, 2, 4, DC))
                xt1 = load_x(xq_d, 1)
                wk = load_w("wk")

                # ---- Phase A: q projection (into resident qT) ----
                for sg in range(2):
                    xt = xt0 if sg == 0 else xt1
                    for ot in range(8):
                        ps = psp.tile([128, 512], f32, tag="pj")
                        for dc in range(DC):
                            nc.tensor.matmul(
                                ps[:],
                                wq[:, dc * D + ot * 128:dc * D + ot * 128 + 128],
                                xt[:, dc * 512:(dc + 1) * 512],
                                start=(dc == 0), stop=(dc == DC - 1))
                        nc.vector.tensor_copy(
                            qT[:, ot * 1024 + sg * 512:ot * 1024 + sg * 512 + 512],
                            ps[:])

                wv = load_w("wv")  # takes wq's slot; prefetches during BC

                # ---- Phase BC: k and v projections from shared x chunks ----
                for sg in range(4):
                    xt = load_x(xkv_d, sg)
                    for ot in range(8):
                        ps = psp.tile([128, 512], f32, tag="pj")
                        for dc in range(DC):
                            nc.tensor.matmul(
                                ps[:],
                                wk[:, dc * D + ot * 128:dc * D + ot * 128 + 128],
                                xt[:, dc * 512:(dc + 1) * 512],
                                start=(dc == 0), stop=(dc == DC - 1))
                        nc.vector.tensor_copy(
                            kT[:, ot * S + sg * 512:ot * S + sg * 512 + 512],
                            ps[:])
                    for st in range(4):
                        for og in range(2):
                            ps = psp.tile([128, 512], f32, tag="pj")
                            for dc in range(DC):
                                nc.tensor.matmul(
                                    ps[:],
                                    xt[:, dc * 512 + st * 128:dc * 512 + st * 128 + 128],
                                    wv[:, dc * D + og * 512:dc * D + og * 512 + 512],
                                    start=(dc == 0), stop=(dc == DC - 1))
                            vtb = ev.tile([128, 512], f32r, tag="ev")
                            nc.vector.tensor_copy(vtb[:], ps[:])
                            nc.sync.dma_start(
                                vtmp[sg][st * 128:st * 128 + 128,
                                         og * 512:og * 512 + 512],
                                vtb[:])
                    if sg == 0:
                        nc.sync.dma_start(
                            vsb0a[:].rearrange("p (c o) -> p c o", c=2),
                            vtmp[0][0:256, :]
                            .rearrange("(c p) o -> p c o", p=128))

            # ---- Phase D: attention ----
            with (
                tc.tile_pool(name="vD", bufs=1) as vp,
                tc.tile_pool(name="cD", bufs=1) as cp,
                tc.tile_pool(name="pD", bufs=2) as pp,
                tc.tile_pool(name="ptD", bufs=1) as ptp,
                tc.tile_pool(name="oD", bufs=1) as op,
                tc.tile_pool(name="smD", bufs=2) as smp,
                tc.tile_pool(name="psS", bufs=3, space="PSUM") as ps_s,
                tc.tile_pool(name="psT", bufs=2, space="PSUM") as ps_t,
                tc.tile_pool(name="psO", bufs=1, space="PSUM") as ps_o,
            ):
                ident = cp.tile([128, 128], f32r)
                nc.sync.dma_start(ident[:], ident_d)
                # vsb[0] holds only blocks 2..3 (0..1 live in vsb0a)
                vsb = [vp.tile([128, (2 if q4 == 0 else 4) * D], f32r,
                               name=f"vsb{q4}", tag=f"v{q4}")
                       for q4 in range(4)]

                def load_v(q4):
                    nb = 2 if q4 == 0 else 4
                    src_rows = vtmp[q4][512 - nb * 128:512, :] \
                        .rearrange("(c p) o -> p c o", p=128)
                    dst = vsb[q4][:].rearrange("p (c o) -> p c o", c=nb)
                    if q4 == 0:
                        for c in range(nb):
                            nc.sync.dma_start(dst[:, c], src_rows[:, c])
                    else:
                        nc.sync.dma_start(dst, src_rows)

                load_v(0)
                masks = cp.tile([128, NT * 512], f32r)  # [p, j*512+kv]
                nc.sync.dma_start(
                    masks[:].rearrange("p (j k) -> p j k", j=NT),
                    masks_d.rearrange("(j p) k -> p j k", p=128))
                for q4 in range(1, 4):
                    load_v(q4)

                def vblk(kb):
                    if kb < 2:
                        return vsb0a[:, kb * D:(kb + 1) * D]
                    if kb < 4:
                        return vsb[0][:, (kb - 2) * D:(kb - 1) * D]
                    return vsb[kb // 4][:, (kb % 4) * D:(kb % 4 + 1) * D]

                Ph, dsh, rch = {}, {}, {}

                def scores_exp(j):
                    ext = _ext(j)
                    ng = (ext + 3) // 4
                    P = pp.tile([128, NB * 128], f32r, tag="P",
                                name=f"P{j}")
                    dslots = smp.tile([128, 4], f32, tag="ds",
                                      name=f"ds{j}")
                    for g in range(ng):
                        gw = min(512, ext * 128 - g * 512)
                        last = (g == ng - 1)
                        sps = ps_s.tile([128, 512], f32, tag="sc",
                                        name=f"sps{j}_{g}")
                        for oc in range(DC):
                            nc.tensor.matmul(
                                sps[:, 0:gw],
                                qT[:, oc * 1024 + j * 128:oc * 1024 + j * 128 + 128],
                                kT[:, oc * S + g * 512:oc * S + g * 512 + gw],
                                start=(oc == 0),
                                stop=(oc == DC - 1 and not last))
                        if last:
                            nc.tensor.matmul(
                                sps[:, 0:gw], ident[:],
                                masks[:, j * 512:j * 512 + gw],
                                start=False, stop=True)
                        nc.scalar.activation(
                            P[:, g * 512:g * 512 + gw], sps[:, 0:gw], AF.Exp,
                            scale=float(SCALE),
                            accum_out=dslots[:, g:g + 1])
                    rcp = smp.tile([128, 1], f32, tag="rcp", name=f"rcp{j}")
                    den = smp.tile([128, 1], f32, tag="den", name=f"den{j}")
                    nc.vector.reduce_sum(den[:], dslots[:, 0:ng],
                                         axis=mybir.AxisListType.X)
                    nc.vector.reciprocal(rcp[:], den[:])
                    Ph[j], dsh[j], rch[j] = P, dslots, rcp

                def transp_av(j):
                    ext = _ext(j)
                    ng = (ext + 3) // 4
                    P, rcp = Ph[j], rch[j]
                    PT = ptp.tile([128, NB * 128], f32r, tag="PT",
                                  name=f"PT{j}")
                    for g in range(ng):
                        nb = min(4, ext - g * 4)
                        tps = ps_t.tile([128, 512], f32r, tag="tp",
                                        name=f"tps{j}_{g}")
                        for bb in range(nb):
                            nc.tensor.transpose(
                                tps[:, bb * 128:(bb + 1) * 128],
                                P[:, g * 512 + bb * 128:g * 512 + bb * 128 + 128],
                                ident[:])
                        nc.vector.tensor_copy(
                            PT[:, g * 512:g * 512 + nb * 128],
                            tps[:, 0:nb * 128])

                    ops = ps_o.tile([128, D], f32, tag="av", name=f"ops{j}")
                    for og in range(2):
                        for kb in range(ext):
                            nc.tensor.matmul(
                                ops[:, og * 512:(og + 1) * 512],
                                PT[:, kb * 128:(kb + 1) * 128],
                                vblk(kb)[:, og * 512:(og + 1) * 512],
                                start=(kb == 0), stop=(kb == ext - 1))
                    osb = op.tile([128, D], f32, tag="o", name=f"o{j}")
                    nc.vector.tensor_scalar_mul(osb[:], ops[:], rcp[:])
                    nc.sync.dma_start(out_d[j * 128:(j + 1) * 128, :], osb[:])

                scores_exp(0)
                for j in range(NT):
                    if j + 1 < NT:
                        scores_exp(j + 1)
                    transp_av(j)

    nc.compile()
    _cache["nc"] = nc
    return nc


def _shard(x, Wq, Wk, Wv):
    """Build the 8 per-core input maps from full inputs."""
    ident = np.eye(128, dtype=np.float32)
    wqt = np.ascontiguousarray(Wq.T)
    wkt = np.ascontiguousarray(Wk.T)
    wvt = np.ascontiguousarray(Wv.T)
    in_maps = []
    for c in range(8):
        b, p = c // 2, c % 2
        xb = x[b]
        xkv = np.ascontiguousarray(xb.T)
        rows = np.concatenate(
            [xb[(2 * j + p) * 128:(2 * j + p + 1) * 128, :] for j in range(8)],
            axis=0)
        xq = np.ascontiguousarray(rows.T)
        masks = np.full((NT * 128, 512), NEG, np.float32)
        for j in range(NT):
            ext = _ext(j)
            ng = (ext + 3) // 4
            gw = min(512, ext * 128 - (ng - 1) * 512)
            q_abs = (2 * j + p) * 128 + np.arange(128)[:, None]
            kv_abs = (ng - 1) * 512 + np.arange(gw)[None, :]
            masks[j * 128:(j + 1) * 128, 0:gw] = np.where(
                kv_abs <= q_abs, np.float32(0), NEG)
        in_maps.append({
            "xq": xq, "xkv": xkv, "wq": wqt, "wk": wkt, "wv": wvt,
            "masks": masks, "ident": ident,
        })
    return in_maps


def _unshard(results, dtype):
    out = np.empty((B, S, D), dtype)
    for c in range(8):
        b, p = c // 2, c % 2
        o = results[c]["out"]
        for j in range(NT):
            out[b, (2 * j + p) * 128:(2 * j + p + 1) * 128, :] = \
                o[j * 128:(j + 1) * 128, :]
    return out


def run(x, Wq, Wk, Wv, trace=False):
    from concourse.bass_utils import run_bass_kernel_spmd
    nc = _build()
    in_maps = _shard(np.asarray(x), np.asarray(Wq), np.asarray(Wk),
                     np.asarray(Wv))
    res = run_bass_kernel_spmd(nc, in_maps, core_ids=list(range(8)),
                               trace=trace)
    return _unshard(res.results, np.float32), res


def kernel(x, Wq, Wk, Wv):
    out, _ = run(x, Wq, Wk, Wv, trace=False)
    return out
